# revision 49
# baseline (speedup 1.0000x reference)
"""Multi-head self-attention (B=2, L=2048, D=1024, H=16, hd=64) on 8 trn2 cores.

Sharding: core c = 4*b + g  (b = batch, g = head-group of 4 heads).
Each core computes Q/K/V projections for its 256 hidden dims (4 heads),
attention for those heads, and a partial output projection
(ctx_g @ Wo[:, g-slice].T + bo/4).  Host sums the 4 partials per batch.

Device algorithm (per core, all fp32):
  - Inputs arrive pre-transposed from host: xT [1024, 2048] (d-major),
    WqT/WkT/WvT [1024, 256], WoT [256, 1024], biases as rows.
  - QT/KT = W.T-projections in [m, L] layout (m on partitions) so that
    S^T = K Q^T comes straight out of the PE per (k-tile, q-bank) with
    k on partitions and q on the free dim. Row-tiled head pairs (K=64).
  - P^T = exp(S^T/8) on the scalar engine (PSUM -> SBUF), grouped 3
    k-tiles per activation op to amortize the ~352-cycle op overhead.
  - ctx^T = [V | 1].T-weighted PV matmul: the appended ones column makes
    PSUM row 64 the softmax denominator for each q.
  - Normalization: recip(denoms) -> rank-1 matmul broadcast -> multiply
    during PSUM evacuation (DVE).
  - Output projection with the bias folded in as a rank-1 (ones x bo/4).
"""

import os
import sys

import numpy as np

for _p in ("/opt/trn_rl_repo", "/root/.axon_site/_ro/trn_rl_repo"):
    if os.path.isdir(_p) and _p not in sys.path:
        sys.path.insert(0, _p)

L = 2048
D = 1024
HD = 64
H_LOC = 4  # heads per core
M_LOC = H_LOC * HD  # 256 hidden dims per core
N_CORES = 8
KT_TILES = L // 128  # 16 k tiles
QB = L // 512  # 4 q banks
DT_TILES = D // 128  # 8 contraction tiles for projections

_PROG = None
_PROG_UNSPLIT = None
LAST_RESULTS = None  # BassKernelResults of the most recent HW run


def _build_program(split=True, reps=1):
    import concourse.bass as bass
    import concourse.mybir as mybir
    import concourse.tile as tile

    fp32 = mybir.dt.float32
    Exp = mybir.ActivationFunctionType.Exp

    # fp32r: PE processes fp32 data in a single full-rate pass (1 cycle/row
    # for moving dim >= 256) instead of fp32's two half-speed passes
    # (4 cycles/row).  Same 4-byte layout; the BIR verifier requires every
    # producer of a matmul operand to carry the float32r dtype.
    R = mybir.dt.float32r
    # bf16 moving operands stream 1 column/cycle at any width (fp32r drops to
    # 2 cycles/column above 256) and enable fast weight loads; all matmul
    # dataflow is bf16 except the softmax-denominator normalize chain.
    bf16 = mybir.dt.bfloat16

    nc = bass.Bass()

    xta = nc.dram_tensor("xta", [D, L], bf16, kind="ExternalInput")
    wqa = nc.dram_tensor("wqa", [D, M_LOC], bf16, kind="ExternalInput")
    wka = nc.dram_tensor("wka", [D, M_LOC], bf16, kind="ExternalInput")
    wva = nc.dram_tensor("wva", [D, M_LOC], bf16, kind="ExternalInput")
    wqb = nc.dram_tensor("wqb", [128, M_LOC], bf16, kind="ExternalInput")
    wkb = nc.dram_tensor("wkb", [128, M_LOC], bf16, kind="ExternalInput")
    woa = nc.dram_tensor("woa", [M_LOC, D], bf16, kind="ExternalInput")
    outp = nc.dram_tensor("outp", [L, D], fp32, kind="ExternalOutput")

    with tile.TileContext(nc) as tc:
        with (
            nc.allow_low_precision(reason="fp32r matmul pipeline; verified vs fp32 reference"),
            tc.tile_pool(name="const", bufs=1) as cpool,
            tc.tile_pool(name="pt", bufs=2) as ptpool,
            tc.tile_pool(name="ev", bufs=2) as epool,
            tc.tile_pool(name="psum", bufs=2, space="PSUM") as psum,
        ):
            # ---- persistent SBUF tiles ----
            wq_t, wk_t, wv_t = [], [], []
            for dt in range(DT_TILES):
                wq_t.append(cpool.tile([128, M_LOC], bf16, tag=f"wq{dt}", name=f"wq{dt}"))
                wk_t.append(cpool.tile([128, M_LOC], bf16, tag=f"wk{dt}", name=f"wk{dt}"))
                wv_t.append(cpool.tile([128, M_LOC], bf16, tag=f"wv{dt}", name=f"wv{dt}"))
            wq1 = cpool.tile([128, M_LOC], bf16, tag="wqbias", name="wqbias")
            wk1 = cpool.tile([128, M_LOC], bf16, tag="wkbias", name="wkbias")
            wo_t = [cpool.tile([128, D], bf16, tag=f"wo{j}", name=f"wo{j}") for j in range(2)]
            qt_t = {}  # (j, lb) -> Q^T [m-tile 128, 512]
            kt_t = {}
            for j in range(2):
                for lb in range(QB):
                    qt_t[(j, lb)] = cpool.tile([128, 512], bf16, tag=f"qt{j}_{lb}", name=f"qt{j}_{lb}")
                    kt_t[(j, lb)] = cpool.tile([128, 512], bf16, tag=f"kt{j}_{lb}", name=f"kt{j}_{lb}")
            # V with appended ones column: [l-part, h, 65]
            v_t = [cpool.tile([128, H_LOC, HD + 1], bf16, tag=f"v{lt}", name=f"v{lt}")
                   for lt in range(KT_TILES)]
            ctxn = {}  # (j, qb) -> normalized ctx^T [128 m, 512 q]
            for j in range(2):
                for qb in range(QB):
                    ctxn[(j, qb)] = cpool.tile([128, 512], bf16, tag=f"cn{j}_{qb}", name=f"cn{j}_{qb}")
            ones = cpool.tile([128, 512], bf16, tag="ones", name="ones")[0:1, :]
            ones_r = cpool.tile([128, HD], R, tag="ones_r", name="ones_r")[0:33, :]
            zfull = cpool.tile([128, HD + 1], bf16, tag="zrow", name="zrow")
            zrow = zfull[0:1, :]
            warm = cpool.tile([128, 8], bf16, tag="warm", name="warm")[0:1, :]

            # ---- input DMAs (weights first, then x^T in lb-major chunks) ----
            nc.sync.dma_start(wq1[:], wqb[:])
            nc.sync.dma_start(wk1[:], wkb[:])
            wq1, wk1 = wq1[0:1, :], wk1[0:1, :]
            for dt in range(DT_TILES):
                nc.sync.dma_start(wq_t[dt][:], wqa[dt * 128:(dt + 1) * 128, :])
                nc.sync.dma_start(wk_t[dt][:], wka[dt * 128:(dt + 1) * 128, :])
                nc.sync.dma_start(wv_t[dt][:], wva[dt * 128:(dt + 1) * 128, :])
            for j in range(2):
                nc.sync.dma_start(wo_t[j][:], woa[j * 128:(j + 1) * 128, :])

            # ---- constants / warmup ----
            nc.gpsimd.memset(ones[:], 1.0)
            nc.gpsimd.memset(ones_r[:].bitcast(fp32), 1.0)
            nc.gpsimd.memset(zfull[:], 0.0)
            for lt in range(KT_TILES):
                nc.gpsimd.memset(v_t[lt][:, :, HD:HD + 1], 1.0)
            # trigger the exp table load early (hides under input DMA)
            nc.scalar.activation(out=warm[:], in_=ones[0:1, 0:8], func=Exp)

            # ---- emission helpers ----
            xt_blocks = {}

            def alloc_xt_block(lb):
                """DMA the 8 d-tiles of one 512-wide L chunk of x^T (kept
                resident so the j=1 projection pass reuses them)."""
                if lb in xt_blocks:
                    return xt_blocks[lb]
                blk = []
                for dt in range(DT_TILES):
                    t = ptpool.tile([128, 512], bf16, tag="xt", name="xtb", bufs=32)
                    nc.sync.dma_start(
                        t[:], xta[dt * 128:(dt + 1) * 128, lb * 512:(lb + 1) * 512]
                    )
                    blk.append(t)
                xt_blocks[lb] = blk
                return blk

            def emit_qk_group(dst, w_tiles, w1, j, xt_blk):
                """dst[m, l] = sum_d W^T[d, m] x^T[d, l] + b[m]  (one q/k bank)."""
                ps = psum.tile([128, 512], fp32, tag="st", name="st")
                for dt in range(DT_TILES):
                    nc.tensor.matmul(
                        ps[:],
                        w_tiles[dt][:, j * 128:(j + 1) * 128],
                        xt_blk[dt][:],
                        start=(dt == 0),
                        stop=False,
                    )
                nc.tensor.matmul(
                    ps[:],
                    w1[0:1, j * 128:(j + 1) * 128],
                    ones[0:1, :],
                    start=False,
                    stop=True,
                )
                nc.vector.tensor_copy(out=dst[:], in_=ps[:])

            def emit_v_group(lt, xt_blk):
                """v_t[lt][l, h, d] = sum_d' x^T[d', l] Wv^T[d', (h,d)].

                No bias: softmax weights sum to 1, so the V bias passes
                through attention exactly and is folded into the host-side
                constant Wo @ bv + bo."""
                li = lt % 4
                ps = psum.tile([128, M_LOC], fp32, tag="st", name="st")
                for dt in range(DT_TILES):
                    nc.tensor.matmul(
                        ps[:],
                        xt_blk[dt][:, li * 128:(li + 1) * 128],
                        wv_t[dt][:],
                        start=(dt == 0),
                        stop=(dt == DT_TILES - 1),
                    )
                nc.vector.tensor_copy(
                    out=v_t[lt][:, :, 0:HD],
                    in_=ps.rearrange("p (h d) -> p h d", d=HD),
                )

            # kt-groups per (j, qb): sizes 3,3,3,3,3,1 (st slot = 3 banks)
            GROUPS = [(0, 3), (3, 3), (6, 3), (9, 3), (12, 3), (15, 1)]

            def emit_attn_group(j, qb, ctx_ab, k0, gn):
                """S^T -> exp -> PV accumulate, for kt in [k0, k0+gn), both heads."""
                sts, pts = [], []
                for hh in range(2):
                    sts.append(psum.tile([128, 3, 512], fp32, tag="st", name="st"))
                # the two heads' K=64 score matmuls occupy disjoint PE row
                # halves; explicit tile_position row groups let them run
                # concurrently (one column pass covers both heads)
                for u in range(gn):
                    kt = k0 + u
                    for hh in range(2):
                        r0, r1 = hh * HD, (hh + 1) * HD
                        nc.tensor.matmul(
                            sts[hh][:, u, :],
                            kt_t[(j, kt // 4)][r0:r1, (kt % 4) * 128:(kt % 4 + 1) * 128],
                            qt_t[(j, qb)][r0:r1, :],
                            start=True,
                            stop=True,
                            tile_position=(hh * HD, 0),
                        )
                for hh in range(2):
                    pt = ptpool.tile([128, 3, 512], bf16, tag="pt", name="pt")
                    pts.append(pt)
                    if os.environ.get("KABL_NOEXP"):
                        nc.vector.tensor_copy(out=pt[:, 0:gn, :], in_=sts[hh][:, 0:gn, :])
                    else:
                        nc.scalar.activation(
                            out=pt[:, 0:gn, :], in_=sts[hh][:, 0:gn, :],
                            func=Exp, scale=0.125,
                        )
                # keep-warm filler: zero-weight matmuls accumulate +0.0 into
                # the live ctx banks.  The attention stream here is paced by
                # the scalar engine's exp; without filler the PE idles in
                # sub-us slivers and the HAM clock drops to 1.2 GHz.
                for _d in range(2):
                    for hh in range(2):
                        nc.tensor.matmul(
                            ctx_ab[hh][:],
                            zfull[:, :],
                            qt_t[(j, qb)][:, :],
                            start=False,
                            stop=False,
                        )
                for u in range(gn):
                    kt = k0 + u
                    for hh in range(2):
                        nc.tensor.matmul(
                            ctx_ab[hh][:],
                            v_t[kt][:, 2 * j + hh, :],
                            pts[hh][:, u, :],
                            start=False,
                            stop=(kt == KT_TILES - 1),
                        )

            # Epilogue is split so the in-order PE queue never drains while
            # the (slow, ~3.3us) DVE reciprocal runs:
            #   pre  — DVE-only: evacuate raw ctx^T+denominator row to SBUF
            #          (frees the ctx PSUM banks) and start the reciprocals.
            #   post — emitted a couple of attention groups later: rank-1
            #          denominator broadcast (PE) + normalize multiply (DVE).
            epi = {}

            def emit_epilogue_pre(j, qb, ctx_ab):
                # both heads' denominator rows staged at partitions 0 and 32
                # (matmul moving operands must start at partition 0/32/64) so
                # ONE batched reciprocal covers them; rows 1..31 are memset so
                # the throwaway lanes stay finite.  The reciprocal goes FIRST
                # on the DVE queue (it gates the deferred rank-1), and the raw
                # ctx evacuations run on the scalar engine, which idles during
                # the epilogue window — keeping the DVE free for the casts and
                # copies the PE is waiting on.
                craws = []
                dstage = epool.tile([33, 512], fp32, tag="dstage",
                                    name="dstage", bufs=4)
                nc.gpsimd.memset(dstage[:], 1.0)
                for hh in range(2):
                    nc.vector.tensor_copy(
                        out=dstage[32 * hh:32 * hh + 1, :],
                        in_=ctx_ab[hh][HD:HD + 1, :],
                    )
                rec = epool.tile([33, 512], R, tag="rec", name="rec", bufs=4)
                nc.vector.reciprocal(rec[:], dstage[:])
                for hh in range(2):
                    craw = epool.tile([HD, 512], fp32, tag="craw",
                                      name="craw", bufs=4)
                    nc.scalar.activation(
                        out=craw[:], in_=ctx_ab[hh][0:HD, :],
                        func=mybir.ActivationFunctionType.Copy,
                    )
                    craws.append(craw)
                epi[(j, qb)] = (craws, rec)

            def emit_epilogue_post(j, qb):
                craws, rec = epi.pop((j, qb))
                rps = []
                for hh in range(2):
                    rp = psum.tile([HD, 512], fp32, tag="st", name="rp")
                    rps.append(rp)
                    for half in range(2):
                        nc.tensor.matmul(
                            rp[:, half * 256:(half + 1) * 256],
                            ones_r[32 * hh:32 * hh + 1, 0:HD],
                            rec[32 * hh:32 * hh + 1, half * 256:(half + 1) * 256],
                            start=True,
                            stop=True,
                        )
                for hh in range(2):
                    nc.vector.tensor_mul(
                        out=ctxn[(j, qb)][hh * HD:(hh + 1) * HD, :],
                        in0=craws[hh][:],
                        in1=rps[hh][:],
                    )

            def emit_oproj_unit(qb, k):
                """One [128 q, 512 d] tile of the output projection — emitted
                individually so the units can be spread between attention
                groups as PE filler while the scalar engine paces exp."""
                qi, nb = k // 2, k % 2
                qt = qb * 4 + qi
                # alternate evacuation engines (scalar/vector) so the PE
                # is never serialized on a single bank's drain.  Tag must be
                # "st": the ctx buffers belong to the in-flight accumulators
                # of the CURRENT qb while these units are interleaved.
                ps = psum.tile([128, 512], fp32, tag="st", name="st")
                for j in range(2):
                    nc.tensor.matmul(
                        ps[:],
                        ctxn[(j, qb)][:, qi * 128:(qi + 1) * 128],
                        wo_t[j][:, nb * 512:(nb + 1) * 512],
                        start=(j == 0),
                        stop=(j == 1),
                    )
                ot = epool.tile([128, 512], fp32, tag="ot", name="ot")
                nc.vector.tensor_copy(out=ot[:], in_=ps[:])
                if not os.environ.get("KABL_NOOUT"):
                    nc.sync.dma_start(
                        outp[qt * 128:(qt + 1) * 128, nb * 512:(nb + 1) * 512],
                        ot[:],
                    )

            def emit_oproj(qb):
                for k in range(8):
                    emit_oproj_unit(qb, k)

            # ---- emission schedule ----
            # lb-progressive j=0 projections with attention (j0, qb0) interleaved
            # so the scalar engine starts exp as early as possible.
            def alloc_ctx():
                ts = [psum.tile([HD + 1, 512], fp32, tag="ctx", name="ctx")
                      for _ in range(2)]
                for t in ts:
                    # rank-1 zeroing matmul opens the accumulation group; it
                    # absorbs the slot-release/bank-drain waits so the first
                    # real PV matmul stays within the PE 2-wait limit
                    nc.tensor.matmul(
                        t[:], zrow[0:1, :], ones[0:1, :], start=True, stop=False
                    )
                return ts

            def lb_block(j, lb, with_v):
                xt_blk = alloc_xt_block(lb)
                emit_qk_group(kt_t[(j, lb)], wk_t, wk1, j, xt_blk)
                emit_qk_group(qt_t[(j, lb)], wq_t, wq1, j, xt_blk)
                if with_v:
                    for lt in range(lb * 4, lb * 4 + 4):
                        emit_v_group(lt, xt_blk)

            for _rep in range(reps):
                xt_blocks.clear()
                ctx00 = alloc_ctx()
                lb_block(0, 0, True)
                emit_attn_group(0, 0, ctx00, *GROUPS[0])
                lb_block(0, 1, True)
                emit_attn_group(0, 0, ctx00, *GROUPS[1])
                lb_block(0, 2, True)
                emit_attn_group(0, 0, ctx00, *GROUPS[2])
                emit_attn_group(0, 0, ctx00, *GROUPS[3])
                lb_block(0, 3, True)
                emit_attn_group(0, 0, ctx00, *GROUPS[4])
                emit_attn_group(0, 0, ctx00, *GROUPS[5])
                emit_epilogue_pre(0, 0, ctx00)

                for qb in range(1, QB):
                    ctx_ab = alloc_ctx()
                    for _d in range(2):
                        for hh in range(2):
                            nc.tensor.matmul(
                                ctx_ab[hh][:], zfull[:, :], qt_t[(0, qb)][:, :],
                                start=False, stop=False,
                            )
                    xt_blk = alloc_xt_block(qb - 1)
                    for g, (k0, gn) in enumerate(GROUPS):
                        emit_attn_group(0, qb, ctx_ab, k0, gn)
                        # j=1 projection chunks spread between groups keep the
                        # PE dense while the scalar engine paces exp
                        if g == 1:
                            emit_qk_group(kt_t[(1, qb - 1)], wk_t, wk1, 1, xt_blk)
                        if g == 2:
                            # previous qb's normalize: its reciprocals have
                            # had ~3 groups of PE work to complete
                            emit_epilogue_post(0, qb - 1)
                        if g == 3:
                            emit_qk_group(qt_t[(1, qb - 1)], wq_t, wq1, 1, xt_blk)
                    emit_epilogue_pre(0, qb, ctx_ab)
                lb_block(1, 3, False)

                for qb in range(QB):
                    ctx_ab = alloc_ctx()
                    for _d in range(3):
                        for hh in range(2):
                            nc.tensor.matmul(
                                ctx_ab[hh][:], zfull[:, :], qt_t[(1, qb)][:, :],
                                start=False, stop=False,
                            )
                    unit_k = 0
                    for g, (k0, gn) in enumerate(GROUPS):
                        emit_attn_group(1, qb, ctx_ab, k0, gn)
                        if g == 2:
                            emit_epilogue_post(1, qb - 1) if qb > 0 else \
                                emit_epilogue_post(0, QB - 1)
                        # previous qb's oproj units spread between groups as
                        # PE filler under the scalar-paced exp stream (only
                        # after its g==2 epilogue_post has produced ctxn)
                        if qb > 0 and g >= 3:
                            emit_oproj_unit(qb - 1, unit_k)
                            unit_k += 1
                            if g >= 4:
                                emit_oproj_unit(qb - 1, unit_k)
                                unit_k += 1
                    while qb > 0 and unit_k < 8:
                        emit_oproj_unit(qb - 1, unit_k)
                        unit_k += 1
                    emit_epilogue_pre(1, qb, ctx_ab)
                emit_epilogue_post(1, QB - 1)
                emit_oproj(QB - 1)

    if split:
        _split_matmul_waits(nc)
    return nc


def _split_matmul_waits(nc):
    """Walrus allows at most 2 sync commands (waits+updates) per PE matmul.

    Move surplus waits onto same-engine NOPs inserted immediately before
    the instruction (engine streams are in-order, so semantics hold).
    """
    import concourse.mybir as mybir

    SPLIT_KINDS = {
        "InstMatmult", "InstDMACopy", "InstActivation", "InstTensorCopy",
        "InstTensorTensor", "InstMemset", "InstReciprocal", "InstTensorReduce",
        "InstTensorScalar", "InstTensorScalarPtr", "InstCopy", "InstDrain",
    }
    nop_id = 0
    for fn in nc.m.functions:
        for bb in fn.blocks:
            insts = bb.instructions
            out = []
            changed = False
            for inst in insts:
                si = getattr(inst, "sync_info", None)
                kind = type(inst).__name__
                budget_total = 1 if kind in ("InstDrain", "InstNoOp") else 2
                if (
                    kind in SPLIT_KINDS
                    and si is not None
                    and si.on_wait
                    and len(si.on_wait) + len(si.on_update or []) > budget_total
                ):
                    budget = budget_total - len(si.on_update or [])
                    keep = si.on_wait[-budget:] if budget > 0 else []
                    surplus = si.on_wait[: len(si.on_wait) - len(keep)]
                    for w in surplus:
                        nop = mybir.InstNoOp(
                            name=f"I-waitnop{nop_id}",
                            engine=inst.engine,
                            ins=[],
                            outs=[],
                            sync_info=mybir.SyncInfo(on_wait=[w], on_update=[]),
                        )
                        nop_id += 1
                        out.append(nop)
                    inst.sync_info = mybir.SyncInfo(
                        on_wait=keep, on_update=si.on_update
                    )
                    changed = True
                out.append(inst)
            if changed:
                bb.instructions = out
    return nc


def _get_program(split=True):
    global _PROG, _PROG_UNSPLIT
    if split:
        if _PROG is None:
            _PROG = _build_program(split=True)
        return _PROG
    if _PROG_UNSPLIT is None:
        _PROG_UNSPLIT = _build_program(split=False)
    return _PROG_UNSPLIT


def _make_in_maps(x, Wq, bq, Wk, bk, Wv, bv, Wo, bo):
    import ml_dtypes

    bf = ml_dtypes.bfloat16
    a = lambda v: np.ascontiguousarray(np.asarray(v, dtype=np.float32).astype(bf))
    in_maps = []
    for c in range(N_CORES):
        b, g = c // 4, c % 4
        s = slice(g * M_LOC, (g + 1) * M_LOC)
        in_maps.append({
            "xta": a(x[b].T),
            "wqa": a(Wq[s, :].T), "wqb": a(np.broadcast_to(bq[s][None, :], (128, M_LOC))),
            "wka": a(Wk[s, :].T), "wkb": a(np.broadcast_to(bk[s][None, :], (128, M_LOC))),
            "wva": a(Wv[s, :].T),
            "woa": a(Wo[:, s].T),
        })
    return in_maps


def _install_ntff_hook():
    """The container's antenv lacks axon_hooks; shim it with the ctypes hook
    from trn_agent_boot so run_bass_kernel_spmd(trace=True) works."""
    import types

    try:
        from antenv.axon_hooks import get_axon_ntff_profile_hook  # noqa
        return
    except ImportError:
        pass
    from trn_agent_boot.trn_boot import _ntff_profile_via_ctypes

    hook = _ntff_profile_via_ctypes("/opt/axon/libaxon_pjrt.so")
    mod = types.ModuleType("antenv.axon_hooks")
    mod.get_axon_ntff_profile_hook = lambda: hook
    mod.set_axon_ntff_profile_hook = lambda h: None
    sys.modules["antenv.axon_hooks"] = mod


def _fake_inputs():
    rng = np.random.default_rng(0)
    return dict(
        x=rng.standard_normal((2, L, D)).astype(np.float32),
        Wq=(rng.standard_normal((D, D)) * 0.03).astype(np.float32),
        bq=(rng.standard_normal(D) * 0.01).astype(np.float32),
        Wk=(rng.standard_normal((D, D)) * 0.03).astype(np.float32),
        bk=(rng.standard_normal(D) * 0.01).astype(np.float32),
        Wv=(rng.standard_normal((D, D)) * 0.03).astype(np.float32),
        bv=(rng.standard_normal(D) * 0.01).astype(np.float32),
        Wo=(rng.standard_normal((D, D)) * 0.03).astype(np.float32),
        bo=(rng.standard_normal(D) * 0.01).astype(np.float32),
    )


def traced_exec_ns(reps=1):
    """Device-side exec time (ns) of the whole program via NTFF profiling.

    No host-timing noise: the NRT profile timestamps the NEFF execution on
    the device itself.
    """
    from concourse import bass_utils

    _install_ntff_hook()
    in_maps = _make_in_maps(**_fake_inputs())
    nc = _build_program(split=True, reps=reps)
    res = bass_utils.run_bass_kernel_spmd(
        nc, in_maps, core_ids=list(range(N_CORES)), trace=True,
    )
    assert res.exec_time_ns is not None, "no NTFF trace captured"
    return res.exec_time_ns, res


def benchmark(reps_a=1, reps_b=5):
    """Steady-state ns per kernel execution: slope of device exec time
    between a reps_a-deep and a reps_b-deep program (cancels cold-clock
    ramp and one-time input DMA)."""
    nsa, _ = traced_exec_ns(reps_a)
    nsb, _ = traced_exec_ns(reps_b)
    return (nsb - nsa) / (reps_b - reps_a)


def kernel(x, Wq, bq, Wk, bk, Wv, bv, Wo, bo):
    global LAST_RESULTS
    x = np.asarray(x, dtype=np.float32)
    nc = _get_program()
    in_maps = _make_in_maps(
        x, np.asarray(Wq), np.asarray(bq), np.asarray(Wk), np.asarray(bk),
        np.asarray(Wv), np.asarray(bv), np.asarray(Wo), np.asarray(bo),
    )

    if os.environ.get("BASS_KERNEL_SIM"):
        from concourse.bass_interp import CoreSim

        nc = _get_program(split=False)
        results = []
        for c in range(int(os.environ.get("BASS_KERNEL_SIM_CORES", N_CORES))):
            sim = CoreSim(nc)
            for name, val in in_maps[c].items():
                sim.tensor(name)[:] = val
            sim.simulate()
            results.append({"outp": np.array(sim.tensor("outp"))})
    else:
        from concourse import bass2jax

        results = bass2jax.run_bass_via_pjrt(nc, in_maps, n_cores=N_CORES)

    B = x.shape[0]
    # V-bias and output bias are linear post-softmax terms: Wo @ bv + bo.
    const = (np.asarray(Wo, dtype=np.float32) @ np.asarray(bv, dtype=np.float32)
             + np.asarray(bo, dtype=np.float32))
    out = np.stack([
        np.sum([results[4 * b + g]["outp"] for g in range(4)], axis=0) + const
        for b in range(B)
    ]).astype(np.float32)
    return out



# revision 51
# speedup vs baseline: 1.0173x; 1.0173x over previous
"""Multi-head self-attention (B=2, L=2048, D=1024, H=16, hd=64) on 8 trn2 cores.

Sharding: core c = 4*b + g  (b = batch, g = head-group of 4 heads).
Each core computes Q/K/V projections for its 256 hidden dims (4 heads),
attention for those heads, and a partial output projection
ctx_g @ Wo[:, g-slice].T.  Host sums the 4 partials per batch and adds the
bias constant Wo @ bv + bo (the V bias passes through softmax exactly, so
it and the output bias are linear post-softmax terms).

Device algorithm (per core):
  - All matmul dataflow is bf16 (host-cast): bf16 moving operands stream
    1 PE column/cycle at any width (fp32 takes 2 half-speed passes; fp32r
    drops to 2 cycles/column above 256) and get fast weight loads.
    Everything accumulates in fp32 PSUM; rel err ~5e-3 vs the gate 2e-2.
  - Inputs arrive pre-transposed from host: xT [1024, 2048] (d-major),
    WqT/WkT/WvT [1024, 256], WoT [256, 1024], biases as rows.
  - QT/KT = W.T-projections in [m, L] layout (m on partitions) so that
    S^T = K Q^T comes straight out of the PE per (k-tile, q-bank) with
    k on partitions and q on the free dim; the two heads of an m-tile are
    K=64 row-halves run concurrently via tile_position row groups.
  - P^T = exp(S^T/8) on the scalar engine (PSUM -> SBUF bf16), grouped 3
    k-tiles per activation op.  The exp stream is the attention-phase
    pacer (~2.7us/group), so oproj units and j=1 projection chunks are
    spread between groups as PE filler, topped up with zero-weight
    "keep-warm" matmuls (accumulate +0.0 into the live ctx banks) --
    without them the PE idles in sub-us slivers and the HAM clock gate
    drops it to 1.2 GHz for ~3.4us stretches.
  - ctx^T = [V | 1].T-weighted PV matmul: the appended ones column makes
    PSUM row 64 the softmax denominator for each q.
  - Epilogue is split so the in-order PE queue never waits on the ~3.3us
    DVE reciprocal: pre (denominator rows staged at partitions 0/32, one
    batched reciprocal, raw ctx evacuated via scalar-engine copies, which
    frees the PSUM banks) and post (fp32r rank-1 denominator broadcast +
    normalize multiply), emitted a few attention groups later.
"""

import os
import sys

import numpy as np

for _p in ("/opt/trn_rl_repo", "/root/.axon_site/_ro/trn_rl_repo"):
    if os.path.isdir(_p) and _p not in sys.path:
        sys.path.insert(0, _p)

L = 2048
D = 1024
HD = 64
H_LOC = 4  # heads per core
M_LOC = H_LOC * HD  # 256 hidden dims per core
N_CORES = 8
KT_TILES = L // 128  # 16 k tiles
QB = L // 512  # 4 q banks
DT_TILES = D // 128  # 8 contraction tiles for projections

_PROG = None
_PROG_UNSPLIT = None
LAST_RESULTS = None  # BassKernelResults of the most recent HW run


def _build_program(split=True, reps=1):
    import concourse.bass as bass
    import concourse.mybir as mybir
    import concourse.tile as tile

    fp32 = mybir.dt.float32
    Exp = mybir.ActivationFunctionType.Exp

    # fp32r: PE processes fp32 data in a single full-rate pass (1 cycle/row
    # for moving dim >= 256) instead of fp32's two half-speed passes
    # (4 cycles/row).  Same 4-byte layout; the BIR verifier requires every
    # producer of a matmul operand to carry the float32r dtype.
    R = mybir.dt.float32r
    # bf16 moving operands stream 1 column/cycle at any width (fp32r drops to
    # 2 cycles/column above 256) and enable fast weight loads; all matmul
    # dataflow is bf16 except the softmax-denominator normalize chain.
    bf16 = mybir.dt.bfloat16

    nc = bass.Bass()

    xta = nc.dram_tensor("xta", [D, L], bf16, kind="ExternalInput")
    wqa = nc.dram_tensor("wqa", [D, M_LOC], bf16, kind="ExternalInput")
    wka = nc.dram_tensor("wka", [D, M_LOC], bf16, kind="ExternalInput")
    wva = nc.dram_tensor("wva", [D, M_LOC], bf16, kind="ExternalInput")
    wqb = nc.dram_tensor("wqb", [128, M_LOC], bf16, kind="ExternalInput")
    wkb = nc.dram_tensor("wkb", [128, M_LOC], bf16, kind="ExternalInput")
    woa = nc.dram_tensor("woa", [M_LOC, D], bf16, kind="ExternalInput")
    outp = nc.dram_tensor("outp", [L, D], fp32, kind="ExternalOutput")

    with tile.TileContext(nc) as tc:
        with (
            nc.allow_low_precision(reason="fp32r matmul pipeline; verified vs fp32 reference"),
            tc.tile_pool(name="const", bufs=1) as cpool,
            tc.tile_pool(name="pt", bufs=2) as ptpool,
            tc.tile_pool(name="ev", bufs=2) as epool,
            tc.tile_pool(name="psum", bufs=2, space="PSUM") as psum,
        ):
            # ---- persistent SBUF tiles ----
            wq_t, wk_t, wv_t = [], [], []
            for dt in range(DT_TILES):
                wq_t.append(cpool.tile([128, M_LOC], bf16, tag=f"wq{dt}", name=f"wq{dt}"))
                wk_t.append(cpool.tile([128, M_LOC], bf16, tag=f"wk{dt}", name=f"wk{dt}"))
                wv_t.append(cpool.tile([128, M_LOC], bf16, tag=f"wv{dt}", name=f"wv{dt}"))
            wq1 = cpool.tile([128, M_LOC], bf16, tag="wqbias", name="wqbias")
            wk1 = cpool.tile([128, M_LOC], bf16, tag="wkbias", name="wkbias")
            wo_t = [cpool.tile([128, D], bf16, tag=f"wo{j}", name=f"wo{j}") for j in range(2)]
            qt_t = {}  # (j, lb) -> Q^T [m-tile 128, 512]
            kt_t = {}
            for j in range(2):
                for lb in range(QB):
                    qt_t[(j, lb)] = cpool.tile([128, 512], bf16, tag=f"qt{j}_{lb}", name=f"qt{j}_{lb}")
                    kt_t[(j, lb)] = cpool.tile([128, 512], bf16, tag=f"kt{j}_{lb}", name=f"kt{j}_{lb}")
            # V with appended ones column: [l-part, h, 65]
            v_t = [cpool.tile([128, H_LOC, HD + 1], bf16, tag=f"v{lt}", name=f"v{lt}")
                   for lt in range(KT_TILES)]
            ctxn = {}  # (j, qb) -> normalized ctx^T [128 m, 512 q]
            for j in range(2):
                for qb in range(QB):
                    ctxn[(j, qb)] = cpool.tile([128, 512], bf16, tag=f"cn{j}_{qb}", name=f"cn{j}_{qb}")
            ones = cpool.tile([128, 512], bf16, tag="ones", name="ones")[0:1, :]
            ones_r = cpool.tile([128, HD], R, tag="ones_r", name="ones_r")[0:33, :]
            zfull = cpool.tile([128, HD + 1], bf16, tag="zrow", name="zrow")
            zrow = zfull[0:1, :]
            warm = cpool.tile([128, 8], bf16, tag="warm", name="warm")[0:1, :]

            # ---- input DMAs (weights first, then x^T in lb-major chunks) ----
            nc.sync.dma_start(wq1[:], wqb[:])
            nc.sync.dma_start(wk1[:], wkb[:])
            wq1, wk1 = wq1[0:1, :], wk1[0:1, :]
            for dt in range(DT_TILES):
                nc.sync.dma_start(wq_t[dt][:], wqa[dt * 128:(dt + 1) * 128, :])
                nc.sync.dma_start(wk_t[dt][:], wka[dt * 128:(dt + 1) * 128, :])
                nc.sync.dma_start(wv_t[dt][:], wva[dt * 128:(dt + 1) * 128, :])
            for j in range(2):
                nc.sync.dma_start(wo_t[j][:], woa[j * 128:(j + 1) * 128, :])

            # ---- constants / warmup ----
            nc.gpsimd.memset(ones[:], 1.0)
            nc.gpsimd.memset(ones_r[:].bitcast(fp32), 1.0)
            nc.gpsimd.memset(zfull[:], 0.0)
            for lt in range(KT_TILES):
                nc.gpsimd.memset(v_t[lt][:, :, HD:HD + 1], 1.0)
            # trigger the exp table load early (hides under input DMA)
            nc.scalar.activation(out=warm[:], in_=ones[0:1, 0:8], func=Exp)

            # ---- emission helpers ----
            xt_blocks = {}

            def alloc_xt_block(lb):
                """DMA the 8 d-tiles of one 512-wide L chunk of x^T (kept
                resident so the j=1 projection pass reuses them)."""
                if lb in xt_blocks:
                    return xt_blocks[lb]
                blk = []
                for dt in range(DT_TILES):
                    t = ptpool.tile([128, 512], bf16, tag="xt", name="xtb", bufs=32)
                    nc.sync.dma_start(
                        t[:], xta[dt * 128:(dt + 1) * 128, lb * 512:(lb + 1) * 512]
                    )
                    blk.append(t)
                xt_blocks[lb] = blk
                return blk

            def emit_qk_group(dst, w_tiles, w1, j, xt_blk):
                """dst[m, l] = sum_d W^T[d, m] x^T[d, l] + b[m]  (one q/k bank)."""
                ps = psum.tile([128, 512], fp32, tag="st", name="st")
                for dt in range(DT_TILES):
                    nc.tensor.matmul(
                        ps[:],
                        w_tiles[dt][:, j * 128:(j + 1) * 128],
                        xt_blk[dt][:],
                        start=(dt == 0),
                        stop=False,
                    )
                nc.tensor.matmul(
                    ps[:],
                    w1[0:1, j * 128:(j + 1) * 128],
                    ones[0:1, :],
                    start=False,
                    stop=True,
                )
                nc.vector.tensor_copy(out=dst[:], in_=ps[:])

            def emit_v_group(lt, xt_blk):
                """v_t[lt][l, h, d] = sum_d' x^T[d', l] Wv^T[d', (h,d)].

                No bias: softmax weights sum to 1, so the V bias passes
                through attention exactly and is folded into the host-side
                constant Wo @ bv + bo."""
                li = lt % 4
                ps = psum.tile([128, M_LOC], fp32, tag="st", name="st")
                for dt in range(DT_TILES):
                    nc.tensor.matmul(
                        ps[:],
                        xt_blk[dt][:, li * 128:(li + 1) * 128],
                        wv_t[dt][:],
                        start=(dt == 0),
                        stop=(dt == DT_TILES - 1),
                    )
                nc.vector.tensor_copy(
                    out=v_t[lt][:, :, 0:HD],
                    in_=ps.rearrange("p (h d) -> p h d", d=HD),
                )

            # kt-groups per (j, qb): sizes 3,3,3,3,3,1 (st slot = 3 banks)
            GROUPS = [(0, 3), (3, 3), (6, 3), (9, 3), (12, 3), (15, 1)]

            def emit_attn_group(j, qb, ctx_ab, k0, gn):
                """S^T -> exp -> PV accumulate, for kt in [k0, k0+gn), both heads."""
                sts, pts = [], []
                for hh in range(2):
                    sts.append(psum.tile([128, 3, 512], fp32, tag="st", name="st"))
                # the two heads' K=64 score matmuls occupy disjoint PE row
                # halves; explicit tile_position row groups let them run
                # concurrently (one column pass covers both heads)
                for u in range(gn):
                    kt = k0 + u
                    for hh in range(2):
                        r0, r1 = hh * HD, (hh + 1) * HD
                        nc.tensor.matmul(
                            sts[hh][:, u, :],
                            kt_t[(j, kt // 4)][r0:r1, (kt % 4) * 128:(kt % 4 + 1) * 128],
                            qt_t[(j, qb)][r0:r1, :],
                            start=True,
                            stop=True,
                            tile_position=(hh * HD, 0),
                        )
                for hh in range(2):
                    pt = ptpool.tile([128, 3, 512], bf16, tag="pt", name="pt")
                    pts.append(pt)
                    if os.environ.get("KABL_NOEXP"):
                        nc.vector.tensor_copy(out=pt[:, 0:gn, :], in_=sts[hh][:, 0:gn, :])
                    else:
                        nc.scalar.activation(
                            out=pt[:, 0:gn, :], in_=sts[hh][:, 0:gn, :],
                            func=Exp, scale=0.125,
                        )
                # keep-warm filler: zero-weight matmuls accumulate +0.0 into
                # the live ctx banks.  The attention stream here is paced by
                # the scalar engine's exp; without filler the PE idles in
                # sub-us slivers and the HAM clock drops to 1.2 GHz.
                for _d in range(2):
                    for hh in range(2):
                        nc.tensor.matmul(
                            ctx_ab[hh][:],
                            zfull[:, :],
                            qt_t[(j, qb)][:, :],
                            start=False,
                            stop=False,
                        )
                for u in range(gn):
                    kt = k0 + u
                    for hh in range(2):
                        nc.tensor.matmul(
                            ctx_ab[hh][:],
                            v_t[kt][:, 2 * j + hh, :],
                            pts[hh][:, u, :],
                            start=False,
                            stop=(kt == KT_TILES - 1),
                        )

            # Epilogue is split so the in-order PE queue never drains while
            # the (slow, ~3.3us) DVE reciprocal runs:
            #   pre  — DVE-only: evacuate raw ctx^T+denominator row to SBUF
            #          (frees the ctx PSUM banks) and start the reciprocals.
            #   post — emitted a couple of attention groups later: rank-1
            #          denominator broadcast (PE) + normalize multiply (DVE).
            epi = {}

            def emit_epilogue_pre(j, qb, ctx_ab):
                # both heads' denominator rows staged at partitions 0 and 32
                # (matmul moving operands must start at partition 0/32/64) so
                # ONE batched reciprocal covers them; rows 1..31 are memset so
                # the throwaway lanes stay finite.  The reciprocal goes FIRST
                # on the DVE queue (it gates the deferred rank-1), and the raw
                # ctx evacuations run on the scalar engine, which idles during
                # the epilogue window — keeping the DVE free for the casts and
                # copies the PE is waiting on.
                craws = []
                dstage = epool.tile([33, 512], fp32, tag="dstage",
                                    name="dstage", bufs=4)
                nc.gpsimd.memset(dstage[:], 1.0)
                for hh in range(2):
                    nc.vector.tensor_copy(
                        out=dstage[32 * hh:32 * hh + 1, :],
                        in_=ctx_ab[hh][HD:HD + 1, :],
                    )
                rec = epool.tile([33, 512], R, tag="rec", name="rec", bufs=4)
                nc.vector.reciprocal(rec[:], dstage[:])
                for hh in range(2):
                    craw = epool.tile([HD, 512], fp32, tag="craw",
                                      name="craw", bufs=4)
                    nc.scalar.activation(
                        out=craw[:], in_=ctx_ab[hh][0:HD, :],
                        func=mybir.ActivationFunctionType.Copy,
                    )
                    craws.append(craw)
                epi[(j, qb)] = (craws, rec)

            def emit_epilogue_post(j, qb):
                craws, rec = epi.pop((j, qb))
                rps = []
                for hh in range(2):
                    rp = psum.tile([HD, 512], fp32, tag="st", name="rp")
                    rps.append(rp)
                    for half in range(2):
                        nc.tensor.matmul(
                            rp[:, half * 256:(half + 1) * 256],
                            ones_r[32 * hh:32 * hh + 1, 0:HD],
                            rec[32 * hh:32 * hh + 1, half * 256:(half + 1) * 256],
                            start=True,
                            stop=True,
                        )
                for hh in range(2):
                    nc.vector.tensor_mul(
                        out=ctxn[(j, qb)][hh * HD:(hh + 1) * HD, :],
                        in0=craws[hh][:],
                        in1=rps[hh][:],
                    )

            def emit_oproj_unit(qb, k):
                """One [128 q, 512 d] tile of the output projection — emitted
                individually so the units can be spread between attention
                groups as PE filler while the scalar engine paces exp."""
                qi, nb = k // 2, k % 2
                qt = qb * 4 + qi
                # alternate evacuation engines (scalar/vector) so the PE
                # is never serialized on a single bank's drain.  Tag must be
                # "st": the ctx buffers belong to the in-flight accumulators
                # of the CURRENT qb while these units are interleaved.
                ps = psum.tile([128, 512], fp32, tag="st", name="st")
                for j in range(2):
                    nc.tensor.matmul(
                        ps[:],
                        ctxn[(j, qb)][:, qi * 128:(qi + 1) * 128],
                        wo_t[j][:, nb * 512:(nb + 1) * 512],
                        start=(j == 0),
                        stop=(j == 1),
                    )
                ot = epool.tile([128, 512], fp32, tag="ot", name="ot")
                nc.vector.tensor_copy(out=ot[:], in_=ps[:])
                if not os.environ.get("KABL_NOOUT"):
                    nc.sync.dma_start(
                        outp[qt * 128:(qt + 1) * 128, nb * 512:(nb + 1) * 512],
                        ot[:],
                    )

            def emit_oproj(qb):
                for k in range(8):
                    emit_oproj_unit(qb, k)

            # ---- emission schedule ----
            # lb-progressive j=0 projections with attention (j0, qb0) interleaved
            # so the scalar engine starts exp as early as possible.
            def alloc_ctx():
                ts = [psum.tile([HD + 1, 512], fp32, tag="ctx", name="ctx")
                      for _ in range(2)]
                for t in ts:
                    # rank-1 zeroing matmul opens the accumulation group; it
                    # absorbs the slot-release/bank-drain waits so the first
                    # real PV matmul stays within the PE 2-wait limit
                    nc.tensor.matmul(
                        t[:], zrow[0:1, :], ones[0:1, :], start=True, stop=False
                    )
                return ts

            def lb_block(j, lb, with_v):
                xt_blk = alloc_xt_block(lb)
                emit_qk_group(kt_t[(j, lb)], wk_t, wk1, j, xt_blk)
                emit_qk_group(qt_t[(j, lb)], wq_t, wq1, j, xt_blk)
                if with_v:
                    for lt in range(lb * 4, lb * 4 + 4):
                        emit_v_group(lt, xt_blk)

            for _rep in range(reps):
                xt_blocks.clear()
                ctx00 = alloc_ctx()
                lb_block(0, 0, True)
                emit_attn_group(0, 0, ctx00, *GROUPS[0])
                lb_block(0, 1, True)
                emit_attn_group(0, 0, ctx00, *GROUPS[1])
                lb_block(0, 2, True)
                emit_attn_group(0, 0, ctx00, *GROUPS[2])
                emit_attn_group(0, 0, ctx00, *GROUPS[3])
                lb_block(0, 3, True)
                emit_attn_group(0, 0, ctx00, *GROUPS[4])
                emit_attn_group(0, 0, ctx00, *GROUPS[5])
                emit_epilogue_pre(0, 0, ctx00)

                for qb in range(1, QB):
                    ctx_ab = alloc_ctx()
                    xt_blk = alloc_xt_block(qb - 1)
                    for g, (k0, gn) in enumerate(GROUPS):
                        emit_attn_group(0, qb, ctx_ab, k0, gn)
                        # j=1 projection chunks spread between groups keep the
                        # PE dense while the scalar engine paces exp
                        if g == 1:
                            emit_qk_group(kt_t[(1, qb - 1)], wk_t, wk1, 1, xt_blk)
                        if g == 2:
                            # previous qb's normalize: its reciprocals have
                            # had ~3 groups of PE work to complete
                            emit_epilogue_post(0, qb - 1)
                        if g == 3:
                            emit_qk_group(qt_t[(1, qb - 1)], wq_t, wq1, 1, xt_blk)
                    emit_epilogue_pre(0, qb, ctx_ab)
                lb_block(1, 3, False)

                for qb in range(QB):
                    ctx_ab = alloc_ctx()
                    for _d in range(3):
                        for hh in range(2):
                            nc.tensor.matmul(
                                ctx_ab[hh][:], zfull[:, :], qt_t[(1, qb)][:, :],
                                start=False, stop=False,
                            )
                    unit_k = 0
                    for g, (k0, gn) in enumerate(GROUPS):
                        emit_attn_group(1, qb, ctx_ab, k0, gn)
                        if g == 2:
                            emit_epilogue_post(1, qb - 1) if qb > 0 else \
                                emit_epilogue_post(0, QB - 1)
                        # previous qb's oproj units spread between groups as
                        # PE filler under the scalar-paced exp stream (only
                        # after its g==2 epilogue_post has produced ctxn)
                        if qb > 0 and g >= 3:
                            emit_oproj_unit(qb - 1, unit_k)
                            unit_k += 1
                            if g >= 4:
                                emit_oproj_unit(qb - 1, unit_k)
                                unit_k += 1
                    while qb > 0 and unit_k < 8:
                        emit_oproj_unit(qb - 1, unit_k)
                        unit_k += 1
                    emit_epilogue_pre(1, qb, ctx_ab)
                emit_epilogue_post(1, QB - 1)
                emit_oproj(QB - 1)

    if split:
        _split_matmul_waits(nc)
    return nc


def _split_matmul_waits(nc):
    """Walrus allows at most 2 sync commands (waits+updates) per PE matmul.

    Move surplus waits onto same-engine NOPs inserted immediately before
    the instruction (engine streams are in-order, so semantics hold).
    """
    import concourse.mybir as mybir

    SPLIT_KINDS = {
        "InstMatmult", "InstDMACopy", "InstActivation", "InstTensorCopy",
        "InstTensorTensor", "InstMemset", "InstReciprocal", "InstTensorReduce",
        "InstTensorScalar", "InstTensorScalarPtr", "InstCopy", "InstDrain",
    }
    nop_id = 0
    for fn in nc.m.functions:
        for bb in fn.blocks:
            insts = bb.instructions
            out = []
            changed = False
            for inst in insts:
                si = getattr(inst, "sync_info", None)
                kind = type(inst).__name__
                budget_total = 1 if kind in ("InstDrain", "InstNoOp") else 2
                if (
                    kind in SPLIT_KINDS
                    and si is not None
                    and si.on_wait
                    and len(si.on_wait) + len(si.on_update or []) > budget_total
                ):
                    budget = budget_total - len(si.on_update or [])
                    keep = si.on_wait[-budget:] if budget > 0 else []
                    surplus = si.on_wait[: len(si.on_wait) - len(keep)]
                    for w in surplus:
                        nop = mybir.InstNoOp(
                            name=f"I-waitnop{nop_id}",
                            engine=inst.engine,
                            ins=[],
                            outs=[],
                            sync_info=mybir.SyncInfo(on_wait=[w], on_update=[]),
                        )
                        nop_id += 1
                        out.append(nop)
                    inst.sync_info = mybir.SyncInfo(
                        on_wait=keep, on_update=si.on_update
                    )
                    changed = True
                out.append(inst)
            if changed:
                bb.instructions = out
    return nc


def _get_program(split=True):
    global _PROG, _PROG_UNSPLIT
    if split:
        if _PROG is None:
            _PROG = _build_program(split=True)
        return _PROG
    if _PROG_UNSPLIT is None:
        _PROG_UNSPLIT = _build_program(split=False)
    return _PROG_UNSPLIT


def _make_in_maps(x, Wq, bq, Wk, bk, Wv, bv, Wo, bo):
    import ml_dtypes

    bf = ml_dtypes.bfloat16
    a = lambda v: np.ascontiguousarray(np.asarray(v, dtype=np.float32).astype(bf))
    in_maps = []
    for c in range(N_CORES):
        b, g = c // 4, c % 4
        s = slice(g * M_LOC, (g + 1) * M_LOC)
        in_maps.append({
            "xta": a(x[b].T),
            "wqa": a(Wq[s, :].T), "wqb": a(np.broadcast_to(bq[s][None, :], (128, M_LOC))),
            "wka": a(Wk[s, :].T), "wkb": a(np.broadcast_to(bk[s][None, :], (128, M_LOC))),
            "wva": a(Wv[s, :].T),
            "woa": a(Wo[:, s].T),
        })
    return in_maps


def _install_ntff_hook():
    """The container's antenv lacks axon_hooks; shim it with the ctypes hook
    from trn_agent_boot so run_bass_kernel_spmd(trace=True) works."""
    import types

    try:
        from antenv.axon_hooks import get_axon_ntff_profile_hook  # noqa
        return
    except ImportError:
        pass
    from trn_agent_boot.trn_boot import _ntff_profile_via_ctypes

    hook = _ntff_profile_via_ctypes("/opt/axon/libaxon_pjrt.so")
    mod = types.ModuleType("antenv.axon_hooks")
    mod.get_axon_ntff_profile_hook = lambda: hook
    mod.set_axon_ntff_profile_hook = lambda h: None
    sys.modules["antenv.axon_hooks"] = mod


def _fake_inputs():
    rng = np.random.default_rng(0)
    return dict(
        x=rng.standard_normal((2, L, D)).astype(np.float32),
        Wq=(rng.standard_normal((D, D)) * 0.03).astype(np.float32),
        bq=(rng.standard_normal(D) * 0.01).astype(np.float32),
        Wk=(rng.standard_normal((D, D)) * 0.03).astype(np.float32),
        bk=(rng.standard_normal(D) * 0.01).astype(np.float32),
        Wv=(rng.standard_normal((D, D)) * 0.03).astype(np.float32),
        bv=(rng.standard_normal(D) * 0.01).astype(np.float32),
        Wo=(rng.standard_normal((D, D)) * 0.03).astype(np.float32),
        bo=(rng.standard_normal(D) * 0.01).astype(np.float32),
    )


def traced_exec_ns(reps=1):
    """Device-side exec time (ns) of the whole program via NTFF profiling.

    No host-timing noise: the NRT profile timestamps the NEFF execution on
    the device itself.
    """
    from concourse import bass_utils

    _install_ntff_hook()
    in_maps = _make_in_maps(**_fake_inputs())
    nc = _build_program(split=True, reps=reps)
    res = bass_utils.run_bass_kernel_spmd(
        nc, in_maps, core_ids=list(range(N_CORES)), trace=True,
    )
    assert res.exec_time_ns is not None, "no NTFF trace captured"
    return res.exec_time_ns, res


def benchmark(reps_a=1, reps_b=5):
    """Steady-state ns per kernel execution: slope of device exec time
    between a reps_a-deep and a reps_b-deep program (cancels cold-clock
    ramp and one-time input DMA)."""
    nsa, _ = traced_exec_ns(reps_a)
    nsb, _ = traced_exec_ns(reps_b)
    return (nsb - nsa) / (reps_b - reps_a)


def kernel(x, Wq, bq, Wk, bk, Wv, bv, Wo, bo):
    global LAST_RESULTS
    x = np.asarray(x, dtype=np.float32)
    nc = _get_program()
    in_maps = _make_in_maps(
        x, np.asarray(Wq), np.asarray(bq), np.asarray(Wk), np.asarray(bk),
        np.asarray(Wv), np.asarray(bv), np.asarray(Wo), np.asarray(bo),
    )

    if os.environ.get("BASS_KERNEL_SIM"):
        from concourse.bass_interp import CoreSim

        nc = _get_program(split=False)
        results = []
        for c in range(int(os.environ.get("BASS_KERNEL_SIM_CORES", N_CORES))):
            sim = CoreSim(nc)
            for name, val in in_maps[c].items():
                sim.tensor(name)[:] = val
            sim.simulate()
            results.append({"outp": np.array(sim.tensor("outp"))})
    else:
        from concourse import bass2jax

        results = bass2jax.run_bass_via_pjrt(nc, in_maps, n_cores=N_CORES)

    B = x.shape[0]
    # V-bias and output bias are linear post-softmax terms: Wo @ bv + bo.
    const = (np.asarray(Wo, dtype=np.float32) @ np.asarray(bv, dtype=np.float32)
             + np.asarray(bo, dtype=np.float32))
    out = np.stack([
        np.sum([results[4 * b + g]["outp"] for g in range(4)], axis=0) + const
        for b in range(B)
    ]).astype(np.float32)
    return out



# revision 52
# speedup vs baseline: 1.0186x; 1.0012x over previous
"""Multi-head self-attention (B=2, L=2048, D=1024, H=16, hd=64) on 8 trn2 cores.

Sharding: core c = 4*b + g  (b = batch, g = head-group of 4 heads).
Each core computes Q/K/V projections for its 256 hidden dims (4 heads),
attention for those heads, and a partial output projection
ctx_g @ Wo[:, g-slice].T.  Host sums the 4 partials per batch and adds the
bias constant Wo @ bv + bo (the V bias passes through softmax exactly, so
it and the output bias are linear post-softmax terms).

Device algorithm (per core):
  - All matmul dataflow is bf16 (host-cast): bf16 moving operands stream
    1 PE column/cycle at any width (fp32 takes 2 half-speed passes; fp32r
    drops to 2 cycles/column above 256) and get fast weight loads.
    Everything accumulates in fp32 PSUM; rel err ~5e-3 vs the gate 2e-2.
  - Inputs arrive pre-transposed from host: xT [1024, 2048] (d-major),
    WqT/WkT/WvT [1024, 256], WoT [256, 1024], biases as rows.
  - QT/KT = W.T-projections in [m, L] layout (m on partitions) so that
    S^T = K Q^T comes straight out of the PE per (k-tile, q-bank) with
    k on partitions and q on the free dim; the two heads of an m-tile are
    K=64 row-halves run concurrently via tile_position row groups.
  - P^T = exp(S^T/8) on the scalar engine (PSUM -> SBUF bf16), grouped 3
    k-tiles per activation op.  The exp stream is the attention-phase
    pacer (~2.7us/group), so oproj units and j=1 projection chunks are
    spread between groups as PE filler, topped up with zero-weight
    "keep-warm" matmuls (accumulate +0.0 into the live ctx banks) --
    without them the PE idles in sub-us slivers and the HAM clock gate
    drops it to 1.2 GHz for ~3.4us stretches.
  - ctx^T = [V | 1].T-weighted PV matmul: the appended ones column makes
    PSUM row 64 the softmax denominator for each q.
  - Epilogue is split so the in-order PE queue never waits on the ~3.3us
    DVE reciprocal: pre (denominator rows staged at partitions 0/32, one
    batched reciprocal, raw ctx evacuated via scalar-engine copies, which
    frees the PSUM banks) and post (fp32r rank-1 denominator broadcast +
    normalize multiply), emitted a few attention groups later.
"""

import os
import sys

import numpy as np

for _p in ("/opt/trn_rl_repo", "/root/.axon_site/_ro/trn_rl_repo"):
    if os.path.isdir(_p) and _p not in sys.path:
        sys.path.insert(0, _p)

L = 2048
D = 1024
HD = 64
H_LOC = 4  # heads per core
M_LOC = H_LOC * HD  # 256 hidden dims per core
N_CORES = 8
KT_TILES = L // 128  # 16 k tiles
QB = L // 512  # 4 q banks
DT_TILES = D // 128  # 8 contraction tiles for projections

_PROG = None
_PROG_UNSPLIT = None
LAST_RESULTS = None  # BassKernelResults of the most recent HW run


def _build_program(split=True, reps=1):
    import concourse.bass as bass
    import concourse.mybir as mybir
    import concourse.tile as tile

    fp32 = mybir.dt.float32
    Exp = mybir.ActivationFunctionType.Exp

    # fp32r: PE processes fp32 data in a single full-rate pass (1 cycle/row
    # for moving dim >= 256) instead of fp32's two half-speed passes
    # (4 cycles/row).  Same 4-byte layout; the BIR verifier requires every
    # producer of a matmul operand to carry the float32r dtype.
    R = mybir.dt.float32r
    # bf16 moving operands stream 1 column/cycle at any width (fp32r drops to
    # 2 cycles/column above 256) and enable fast weight loads; all matmul
    # dataflow is bf16 except the softmax-denominator normalize chain.
    bf16 = mybir.dt.bfloat16

    nc = bass.Bass()

    xta = nc.dram_tensor("xta", [D, L], bf16, kind="ExternalInput")
    wqa = nc.dram_tensor("wqa", [D, M_LOC], bf16, kind="ExternalInput")
    wka = nc.dram_tensor("wka", [D, M_LOC], bf16, kind="ExternalInput")
    wva = nc.dram_tensor("wva", [D, M_LOC], bf16, kind="ExternalInput")
    wqb = nc.dram_tensor("wqb", [128, M_LOC], bf16, kind="ExternalInput")
    wkb = nc.dram_tensor("wkb", [128, M_LOC], bf16, kind="ExternalInput")
    woa = nc.dram_tensor("woa", [M_LOC, D], bf16, kind="ExternalInput")
    outp = nc.dram_tensor("outp", [L, D], fp32, kind="ExternalOutput")

    with tile.TileContext(nc) as tc:
        with (
            nc.allow_low_precision(reason="fp32r matmul pipeline; verified vs fp32 reference"),
            tc.tile_pool(name="const", bufs=1) as cpool,
            tc.tile_pool(name="pt", bufs=2) as ptpool,
            tc.tile_pool(name="ev", bufs=2) as epool,
            tc.tile_pool(name="psum", bufs=2, space="PSUM") as psum,
        ):
            # ---- persistent SBUF tiles ----
            wq_t, wk_t, wv_t = [], [], []
            for dt in range(DT_TILES):
                wq_t.append(cpool.tile([128, M_LOC], bf16, tag=f"wq{dt}", name=f"wq{dt}"))
                wk_t.append(cpool.tile([128, M_LOC], bf16, tag=f"wk{dt}", name=f"wk{dt}"))
                wv_t.append(cpool.tile([128, M_LOC], bf16, tag=f"wv{dt}", name=f"wv{dt}"))
            wq1 = cpool.tile([128, M_LOC], bf16, tag="wqbias", name="wqbias")
            wk1 = cpool.tile([128, M_LOC], bf16, tag="wkbias", name="wkbias")
            wo_t = [cpool.tile([128, D], bf16, tag=f"wo{j}", name=f"wo{j}") for j in range(2)]
            qt_t = {}  # (j, lb) -> Q^T [m-tile 128, 512]
            kt_t = {}
            for j in range(2):
                for lb in range(QB):
                    qt_t[(j, lb)] = cpool.tile([128, 512], bf16, tag=f"qt{j}_{lb}", name=f"qt{j}_{lb}")
                    kt_t[(j, lb)] = cpool.tile([128, 512], bf16, tag=f"kt{j}_{lb}", name=f"kt{j}_{lb}")
            # V with appended ones column: [l-part, h, 65]
            v_t = [cpool.tile([128, H_LOC, HD + 1], bf16, tag=f"v{lt}", name=f"v{lt}")
                   for lt in range(KT_TILES)]
            ctxn = {}  # (j, qb) -> normalized ctx^T [128 m, 512 q]
            for j in range(2):
                for qb in range(QB):
                    ctxn[(j, qb)] = cpool.tile([128, 512], bf16, tag=f"cn{j}_{qb}", name=f"cn{j}_{qb}")
            ones = cpool.tile([128, 512], bf16, tag="ones", name="ones")[0:1, :]
            ones_r = cpool.tile([128, HD], R, tag="ones_r", name="ones_r")[0:33, :]
            zfull = cpool.tile([128, HD + 1], bf16, tag="zrow", name="zrow")
            zrow = zfull[0:1, :]
            warm = cpool.tile([128, 8], bf16, tag="warm", name="warm")[0:1, :]

            # ---- input DMAs (weights first, then x^T in lb-major chunks) ----
            nc.sync.dma_start(wq1[:], wqb[:])
            nc.sync.dma_start(wk1[:], wkb[:])
            wq1, wk1 = wq1[0:1, :], wk1[0:1, :]
            for dt in range(DT_TILES):
                nc.sync.dma_start(wq_t[dt][:], wqa[dt * 128:(dt + 1) * 128, :])
                nc.sync.dma_start(wk_t[dt][:], wka[dt * 128:(dt + 1) * 128, :])
                nc.sync.dma_start(wv_t[dt][:], wva[dt * 128:(dt + 1) * 128, :])
            for j in range(2):
                nc.sync.dma_start(wo_t[j][:], woa[j * 128:(j + 1) * 128, :])

            # ---- constants / warmup ----
            nc.gpsimd.memset(ones[:], 1.0)
            nc.gpsimd.memset(ones_r[:].bitcast(fp32), 1.0)
            nc.gpsimd.memset(zfull[:], 0.0)
            for lt in range(KT_TILES):
                nc.gpsimd.memset(v_t[lt][:, :, HD:HD + 1], 1.0)
            # trigger the exp table load early (hides under input DMA)
            nc.scalar.activation(out=warm[:], in_=ones[0:1, 0:8], func=Exp)

            # ---- emission helpers ----
            xt_blocks = {}

            def alloc_xt_block(lb):
                """DMA the 8 d-tiles of one 512-wide L chunk of x^T (kept
                resident so the j=1 projection pass reuses them)."""
                if lb in xt_blocks:
                    return xt_blocks[lb]
                blk = []
                for dt in range(DT_TILES):
                    t = ptpool.tile([128, 512], bf16, tag="xt", name="xtb", bufs=32)
                    nc.sync.dma_start(
                        t[:], xta[dt * 128:(dt + 1) * 128, lb * 512:(lb + 1) * 512]
                    )
                    blk.append(t)
                xt_blocks[lb] = blk
                return blk

            def emit_qk_group(dst, w_tiles, w1, j, xt_blk):
                """dst[m, l] = sum_d W^T[d, m] x^T[d, l] + b[m]  (one q/k bank)."""
                ps = psum.tile([128, 512], fp32, tag="st", name="st")
                for dt in range(DT_TILES):
                    nc.tensor.matmul(
                        ps[:],
                        w_tiles[dt][:, j * 128:(j + 1) * 128],
                        xt_blk[dt][:],
                        start=(dt == 0),
                        stop=False,
                    )
                nc.tensor.matmul(
                    ps[:],
                    w1[0:1, j * 128:(j + 1) * 128],
                    ones[0:1, :],
                    start=False,
                    stop=True,
                )
                nc.vector.tensor_copy(out=dst[:], in_=ps[:])

            def emit_v_group(lt, xt_blk):
                """v_t[lt][l, h, d] = sum_d' x^T[d', l] Wv^T[d', (h,d)].

                No bias: softmax weights sum to 1, so the V bias passes
                through attention exactly and is folded into the host-side
                constant Wo @ bv + bo."""
                li = lt % 4
                ps = psum.tile([128, M_LOC], fp32, tag="st", name="st")
                for dt in range(DT_TILES):
                    nc.tensor.matmul(
                        ps[:],
                        xt_blk[dt][:, li * 128:(li + 1) * 128],
                        wv_t[dt][:],
                        start=(dt == 0),
                        stop=(dt == DT_TILES - 1),
                    )
                nc.vector.tensor_copy(
                    out=v_t[lt][:, :, 0:HD],
                    in_=ps.rearrange("p (h d) -> p h d", d=HD),
                )

            # kt-groups per (j, qb): sizes 3,3,3,3,3,1 (st slot = 3 banks)
            GROUPS = [(0, 3), (3, 3), (6, 3), (9, 3), (12, 3), (15, 1)]

            def emit_attn_group(j, qb, ctx_ab, k0, gn):
                """S^T -> exp -> PV accumulate, for kt in [k0, k0+gn), both heads."""
                sts, pts = [], []
                for hh in range(2):
                    sts.append(psum.tile([128, 3, 512], fp32, tag="st", name="st"))
                # the two heads' K=64 score matmuls occupy disjoint PE row
                # halves; explicit tile_position row groups let them run
                # concurrently (one column pass covers both heads)
                for u in range(gn):
                    kt = k0 + u
                    for hh in range(2):
                        r0, r1 = hh * HD, (hh + 1) * HD
                        nc.tensor.matmul(
                            sts[hh][:, u, :],
                            kt_t[(j, kt // 4)][r0:r1, (kt % 4) * 128:(kt % 4 + 1) * 128],
                            qt_t[(j, qb)][r0:r1, :],
                            start=True,
                            stop=True,
                            tile_position=(hh * HD, 0),
                        )
                for hh in range(2):
                    pt = ptpool.tile([128, 3, 512], bf16, tag="pt", name="pt")
                    pts.append(pt)
                    if os.environ.get("KABL_NOEXP"):
                        nc.vector.tensor_copy(out=pt[:, 0:gn, :], in_=sts[hh][:, 0:gn, :])
                    else:
                        nc.scalar.activation(
                            out=pt[:, 0:gn, :], in_=sts[hh][:, 0:gn, :],
                            func=Exp, scale=0.125,
                        )
                # keep-warm filler: zero-weight matmuls accumulate +0.0 into
                # the live ctx banks.  The attention stream here is paced by
                # the scalar engine's exp; without filler the PE idles in
                # sub-us slivers and the HAM clock drops to 1.2 GHz.
                for _d in range(2):
                    for hh in range(2):
                        nc.tensor.matmul(
                            ctx_ab[hh][:],
                            zfull[:, :],
                            qt_t[(j, qb)][:, :],
                            start=False,
                            stop=False,
                        )
                for u in range(gn):
                    kt = k0 + u
                    for hh in range(2):
                        nc.tensor.matmul(
                            ctx_ab[hh][:],
                            v_t[kt][:, 2 * j + hh, :],
                            pts[hh][:, u, :],
                            start=False,
                            stop=(kt == KT_TILES - 1),
                        )

            # Epilogue is split so the in-order PE queue never drains while
            # the (slow, ~3.3us) DVE reciprocal runs:
            #   pre  — DVE-only: evacuate raw ctx^T+denominator row to SBUF
            #          (frees the ctx PSUM banks) and start the reciprocals.
            #   post — emitted a couple of attention groups later: rank-1
            #          denominator broadcast (PE) + normalize multiply (DVE).
            epi = {}

            def emit_epilogue_pre(j, qb, ctx_ab):
                # both heads' denominator rows staged at partitions 0 and 32
                # (matmul moving operands must start at partition 0/32/64) so
                # ONE batched reciprocal covers them; rows 1..31 are memset so
                # the throwaway lanes stay finite.  The reciprocal goes FIRST
                # on the DVE queue (it gates the deferred rank-1), and the raw
                # ctx evacuations run on the scalar engine, which idles during
                # the epilogue window — keeping the DVE free for the casts and
                # copies the PE is waiting on.
                craws = []
                dstage = epool.tile([33, 512], fp32, tag="dstage",
                                    name="dstage", bufs=4)
                nc.gpsimd.memset(dstage[:], 1.0)
                for hh in range(2):
                    nc.vector.tensor_copy(
                        out=dstage[32 * hh:32 * hh + 1, :],
                        in_=ctx_ab[hh][HD:HD + 1, :],
                    )
                rec = epool.tile([33, 512], R, tag="rec", name="rec", bufs=4)
                nc.vector.reciprocal(rec[:], dstage[:])
                for hh in range(2):
                    craw = epool.tile([HD, 512], fp32, tag="craw",
                                      name="craw", bufs=4)
                    nc.scalar.activation(
                        out=craw[:], in_=ctx_ab[hh][0:HD, :],
                        func=mybir.ActivationFunctionType.Copy,
                    )
                    craws.append(craw)
                epi[(j, qb)] = (craws, rec)

            def emit_epilogue_post(j, qb):
                craws, rec = epi.pop((j, qb))
                rps = []
                for hh in range(2):
                    rp = psum.tile([HD, 512], fp32, tag="st", name="rp")
                    rps.append(rp)
                    for half in range(2):
                        nc.tensor.matmul(
                            rp[:, half * 256:(half + 1) * 256],
                            ones_r[32 * hh:32 * hh + 1, 0:HD],
                            rec[32 * hh:32 * hh + 1, half * 256:(half + 1) * 256],
                            start=True,
                            stop=True,
                        )
                for hh in range(2):
                    nc.vector.tensor_mul(
                        out=ctxn[(j, qb)][hh * HD:(hh + 1) * HD, :],
                        in0=craws[hh][:],
                        in1=rps[hh][:],
                    )

            def emit_oproj_unit(qb, k):
                """One [128 q, 512 d] tile of the output projection — emitted
                individually so the units can be spread between attention
                groups as PE filler while the scalar engine paces exp."""
                qi, nb = k // 2, k % 2
                qt = qb * 4 + qi
                # alternate evacuation engines (scalar/vector) so the PE
                # is never serialized on a single bank's drain.  Tag must be
                # "st": the ctx buffers belong to the in-flight accumulators
                # of the CURRENT qb while these units are interleaved.
                ps = psum.tile([128, 512], fp32, tag="st", name="st")
                for j in range(2):
                    nc.tensor.matmul(
                        ps[:],
                        ctxn[(j, qb)][:, qi * 128:(qi + 1) * 128],
                        wo_t[j][:, nb * 512:(nb + 1) * 512],
                        start=(j == 0),
                        stop=(j == 1),
                    )
                ot = epool.tile([128, 512], fp32, tag="ot", name="ot")
                nc.vector.tensor_copy(out=ot[:], in_=ps[:])
                if not os.environ.get("KABL_NOOUT"):
                    nc.sync.dma_start(
                        outp[qt * 128:(qt + 1) * 128, nb * 512:(nb + 1) * 512],
                        ot[:],
                    )

            def emit_oproj(qb):
                for k in range(8):
                    emit_oproj_unit(qb, k)

            # ---- emission schedule ----
            # lb-progressive j=0 projections with attention (j0, qb0) interleaved
            # so the scalar engine starts exp as early as possible.
            def alloc_ctx():
                ts = [psum.tile([HD + 1, 512], fp32, tag="ctx", name="ctx")
                      for _ in range(2)]
                for t in ts:
                    # rank-1 zeroing matmul opens the accumulation group; it
                    # absorbs the slot-release/bank-drain waits so the first
                    # real PV matmul stays within the PE 2-wait limit
                    nc.tensor.matmul(
                        t[:], zrow[0:1, :], ones[0:1, :], start=True, stop=False
                    )
                return ts

            def lb_block(j, lb, with_v):
                xt_blk = alloc_xt_block(lb)
                emit_qk_group(kt_t[(j, lb)], wk_t, wk1, j, xt_blk)
                emit_qk_group(qt_t[(j, lb)], wq_t, wq1, j, xt_blk)
                if with_v:
                    for lt in range(lb * 4, lb * 4 + 4):
                        emit_v_group(lt, xt_blk)

            pending_tail = False
            for _rep in range(reps):
                xt_blocks.clear()
                ctx00 = alloc_ctx()
                lb_block(0, 0, True)
                emit_attn_group(0, 0, ctx00, *GROUPS[0])
                lb_block(0, 1, True)
                emit_attn_group(0, 0, ctx00, *GROUPS[1])
                if pending_tail:
                    # previous rep's last normalize, deferred across the rep
                    # boundary so its reciprocals hide under this rep's start
                    emit_epilogue_post(1, QB - 1)
                lb_block(0, 2, True)
                emit_attn_group(0, 0, ctx00, *GROUPS[2])
                emit_attn_group(0, 0, ctx00, *GROUPS[3])
                if pending_tail:
                    emit_oproj(QB - 1)
                    pending_tail = False
                lb_block(0, 3, True)
                emit_attn_group(0, 0, ctx00, *GROUPS[4])
                emit_attn_group(0, 0, ctx00, *GROUPS[5])
                emit_epilogue_pre(0, 0, ctx00)

                for qb in range(1, QB):
                    ctx_ab = alloc_ctx()
                    xt_blk = alloc_xt_block(qb - 1)
                    for g, (k0, gn) in enumerate(GROUPS):
                        emit_attn_group(0, qb, ctx_ab, k0, gn)
                        # j=1 projection chunks spread between groups keep the
                        # PE dense while the scalar engine paces exp
                        if g == 1:
                            emit_qk_group(kt_t[(1, qb - 1)], wk_t, wk1, 1, xt_blk)
                        if g == 2:
                            # previous qb's normalize: its reciprocals have
                            # had ~3 groups of PE work to complete
                            emit_epilogue_post(0, qb - 1)
                        if g == 3:
                            emit_qk_group(qt_t[(1, qb - 1)], wq_t, wq1, 1, xt_blk)
                    emit_epilogue_pre(0, qb, ctx_ab)
                lb_block(1, 3, False)

                for qb in range(QB):
                    ctx_ab = alloc_ctx()
                    for _d in range(3):
                        for hh in range(2):
                            nc.tensor.matmul(
                                ctx_ab[hh][:], zfull[:, :], qt_t[(1, qb)][:, :],
                                start=False, stop=False,
                            )
                    unit_k = 0
                    for g, (k0, gn) in enumerate(GROUPS):
                        emit_attn_group(1, qb, ctx_ab, k0, gn)
                        if g == 2:
                            emit_epilogue_post(1, qb - 1) if qb > 0 else \
                                emit_epilogue_post(0, QB - 1)
                        # previous qb's oproj units spread between groups as
                        # PE filler under the scalar-paced exp stream (only
                        # after its g==2 epilogue_post has produced ctxn)
                        if qb > 0 and g >= 3:
                            emit_oproj_unit(qb - 1, unit_k)
                            unit_k += 1
                            if g >= 4:
                                emit_oproj_unit(qb - 1, unit_k)
                                unit_k += 1
                    while qb > 0 and unit_k < 8:
                        emit_oproj_unit(qb - 1, unit_k)
                        unit_k += 1
                    emit_epilogue_pre(1, qb, ctx_ab)
                pending_tail = True
            emit_epilogue_post(1, QB - 1)
            emit_oproj(QB - 1)

    if split:
        _split_matmul_waits(nc)
    return nc


def _split_matmul_waits(nc):
    """Walrus allows at most 2 sync commands (waits+updates) per PE matmul.

    Move surplus waits onto same-engine NOPs inserted immediately before
    the instruction (engine streams are in-order, so semantics hold).
    """
    import concourse.mybir as mybir

    SPLIT_KINDS = {
        "InstMatmult", "InstDMACopy", "InstActivation", "InstTensorCopy",
        "InstTensorTensor", "InstMemset", "InstReciprocal", "InstTensorReduce",
        "InstTensorScalar", "InstTensorScalarPtr", "InstCopy", "InstDrain",
    }
    nop_id = 0
    for fn in nc.m.functions:
        for bb in fn.blocks:
            insts = bb.instructions
            out = []
            changed = False
            for inst in insts:
                si = getattr(inst, "sync_info", None)
                kind = type(inst).__name__
                budget_total = 1 if kind in ("InstDrain", "InstNoOp") else 2
                if (
                    kind in SPLIT_KINDS
                    and si is not None
                    and si.on_wait
                    and len(si.on_wait) + len(si.on_update or []) > budget_total
                ):
                    budget = budget_total - len(si.on_update or [])
                    keep = si.on_wait[-budget:] if budget > 0 else []
                    surplus = si.on_wait[: len(si.on_wait) - len(keep)]
                    for w in surplus:
                        nop = mybir.InstNoOp(
                            name=f"I-waitnop{nop_id}",
                            engine=inst.engine,
                            ins=[],
                            outs=[],
                            sync_info=mybir.SyncInfo(on_wait=[w], on_update=[]),
                        )
                        nop_id += 1
                        out.append(nop)
                    inst.sync_info = mybir.SyncInfo(
                        on_wait=keep, on_update=si.on_update
                    )
                    changed = True
                out.append(inst)
            if changed:
                bb.instructions = out
    return nc


def _get_program(split=True):
    global _PROG, _PROG_UNSPLIT
    if split:
        if _PROG is None:
            _PROG = _build_program(split=True)
        return _PROG
    if _PROG_UNSPLIT is None:
        _PROG_UNSPLIT = _build_program(split=False)
    return _PROG_UNSPLIT


def _make_in_maps(x, Wq, bq, Wk, bk, Wv, bv, Wo, bo):
    import ml_dtypes

    bf = ml_dtypes.bfloat16
    a = lambda v: np.ascontiguousarray(np.asarray(v, dtype=np.float32).astype(bf))
    in_maps = []
    for c in range(N_CORES):
        b, g = c // 4, c % 4
        s = slice(g * M_LOC, (g + 1) * M_LOC)
        in_maps.append({
            "xta": a(x[b].T),
            "wqa": a(Wq[s, :].T), "wqb": a(np.broadcast_to(bq[s][None, :], (128, M_LOC))),
            "wka": a(Wk[s, :].T), "wkb": a(np.broadcast_to(bk[s][None, :], (128, M_LOC))),
            "wva": a(Wv[s, :].T),
            "woa": a(Wo[:, s].T),
        })
    return in_maps


def _install_ntff_hook():
    """The container's antenv lacks axon_hooks; shim it with the ctypes hook
    from trn_agent_boot so run_bass_kernel_spmd(trace=True) works."""
    import types

    try:
        from antenv.axon_hooks import get_axon_ntff_profile_hook  # noqa
        return
    except ImportError:
        pass
    from trn_agent_boot.trn_boot import _ntff_profile_via_ctypes

    hook = _ntff_profile_via_ctypes("/opt/axon/libaxon_pjrt.so")
    mod = types.ModuleType("antenv.axon_hooks")
    mod.get_axon_ntff_profile_hook = lambda: hook
    mod.set_axon_ntff_profile_hook = lambda h: None
    sys.modules["antenv.axon_hooks"] = mod


def _fake_inputs():
    rng = np.random.default_rng(0)
    return dict(
        x=rng.standard_normal((2, L, D)).astype(np.float32),
        Wq=(rng.standard_normal((D, D)) * 0.03).astype(np.float32),
        bq=(rng.standard_normal(D) * 0.01).astype(np.float32),
        Wk=(rng.standard_normal((D, D)) * 0.03).astype(np.float32),
        bk=(rng.standard_normal(D) * 0.01).astype(np.float32),
        Wv=(rng.standard_normal((D, D)) * 0.03).astype(np.float32),
        bv=(rng.standard_normal(D) * 0.01).astype(np.float32),
        Wo=(rng.standard_normal((D, D)) * 0.03).astype(np.float32),
        bo=(rng.standard_normal(D) * 0.01).astype(np.float32),
    )


def traced_exec_ns(reps=1):
    """Device-side exec time (ns) of the whole program via NTFF profiling.

    No host-timing noise: the NRT profile timestamps the NEFF execution on
    the device itself.
    """
    from concourse import bass_utils

    _install_ntff_hook()
    in_maps = _make_in_maps(**_fake_inputs())
    nc = _build_program(split=True, reps=reps)
    res = bass_utils.run_bass_kernel_spmd(
        nc, in_maps, core_ids=list(range(N_CORES)), trace=True,
    )
    assert res.exec_time_ns is not None, "no NTFF trace captured"
    return res.exec_time_ns, res


def benchmark(reps_a=1, reps_b=5):
    """Steady-state ns per kernel execution: slope of device exec time
    between a reps_a-deep and a reps_b-deep program (cancels cold-clock
    ramp and one-time input DMA)."""
    nsa, _ = traced_exec_ns(reps_a)
    nsb, _ = traced_exec_ns(reps_b)
    return (nsb - nsa) / (reps_b - reps_a)


def kernel(x, Wq, bq, Wk, bk, Wv, bv, Wo, bo):
    global LAST_RESULTS
    x = np.asarray(x, dtype=np.float32)
    nc = _get_program()
    in_maps = _make_in_maps(
        x, np.asarray(Wq), np.asarray(bq), np.asarray(Wk), np.asarray(bk),
        np.asarray(Wv), np.asarray(bv), np.asarray(Wo), np.asarray(bo),
    )

    if os.environ.get("BASS_KERNEL_SIM"):
        from concourse.bass_interp import CoreSim

        nc = _get_program(split=False)
        results = []
        for c in range(int(os.environ.get("BASS_KERNEL_SIM_CORES", N_CORES))):
            sim = CoreSim(nc)
            for name, val in in_maps[c].items():
                sim.tensor(name)[:] = val
            sim.simulate()
            results.append({"outp": np.array(sim.tensor("outp"))})
    else:
        from concourse import bass2jax

        results = bass2jax.run_bass_via_pjrt(nc, in_maps, n_cores=N_CORES)

    B = x.shape[0]
    # V-bias and output bias are linear post-softmax terms: Wo @ bv + bo.
    const = (np.asarray(Wo, dtype=np.float32) @ np.asarray(bv, dtype=np.float32)
             + np.asarray(bo, dtype=np.float32))
    out = np.stack([
        np.sum([results[4 * b + g]["outp"] for g in range(4)], axis=0) + const
        for b in range(B)
    ]).astype(np.float32)
    return out

